# revision 11
# baseline (speedup 1.0000x reference)
"""BiLSTM-CRF loss on 8 Trainium2 NeuronCores — full device implementation.

Per core (8 sequences): 2-layer BiLSTM (input projections as GEMMs + fused
f/b recurrence), LayerNorm + emission GEMM, CRF forward pass via the
exp-transition matmul trick. Host does only: embedding gather/transpose,
weight pre-transposes, CRF real-path score (gather-heavy), final mean.

Layout notes:
- "rows" are t-major: row = t*8 + b_local (t in [0,512), b_local in [0,8)).
- Recurrence tiles hold fwd dir in partitions 0:8 and bwd dir in 32:40
  (compute-engine partition bases must be 32-aligned; rows 8:32 are junk).
- Backward direction runs on reversed time internally; all reversals are
  static index arithmetic, no reversed DMAs.
"""

import os
import sys

os.environ.setdefault("JAX_COMPILATION_CACHE_DIR", "/root/.cache/jax_bass_cache")
os.environ.setdefault("JAX_PERSISTENT_CACHE_MIN_ENTRY_SIZE_BYTES", "-1")
os.environ.setdefault("JAX_PERSISTENT_CACHE_MIN_COMPILE_TIME_SECS", "0")

import numpy as np

for _p in ("/opt/trn_rl_repo",):
    if _p not in sys.path:
        sys.path.append(_p)

def _install_bir_scrub():
    """Make serialized BIR independent of this file's location: debug-info
    filenames otherwise change the compiler cache key per directory."""
    import concourse.bass as bass
    if getattr(bass.Bass, "_bir_scrubbed", False):
        return
    orig = bass.Bass.to_json_bytes
    here = os.path.abspath(__file__).encode()

    def scrubbed(self):
        return orig(self).replace(here, b"/k.py")

    bass.Bass.to_json_bytes = scrubbed
    bass.Bass._bir_scrubbed = True


B, T, V, E, D, K = 64, 512, 8000, 128, 512, 35
H = D // 2
START, STOP = 33, 34
NEG = -10000.0
NCORES = 8
BC = B // NCORES  # 8
ROWS = BC * T  # 4096
CHUNK = 8
NCHUNK = T // CHUNK  # 64

_nc_cache = None


def _split_waits(nc):
    """Hoist extra sync waits onto NoOps: this walrus build rejects
    instructions carrying more than one sync wait. Engines execute their
    stream in order, so a preceding same-engine NoOp wait is equivalent."""
    from concourse import mybir
    n_split = 0
    for func in nc.m.functions:
        for block in func.blocks:
            out = []
            changed = False
            for inst in block.instructions:
                si = inst.sync_info
                waits = list(si.on_wait) if si is not None else []
                if len(waits) > 1:
                    changed = True
                    for w in waits[:-1]:
                        n_split += 1
                        out.append(mybir.InstNoOp(
                            name=f"{inst.name}-wsplit{n_split}",
                            engine=inst.engine, ins=[], outs=[],
                            sync_info=mybir.SyncInfo(on_wait=[w], on_update=[])))
                    si.on_wait = waits[-1:]
                out.append(inst)
            if changed:
                block.instructions = out
    # Strip source-path debug info so the serialized BIR (and thus the
    # compiler cache key) is independent of where kernel.py lives.
    for func in nc.m.functions:
        for block in func.blocks:
            for inst in block.instructions:
                try:
                    inst.debug = None
                except Exception:
                    pass
                try:
                    inst.bass_addl_debug = None
                except Exception:
                    pass
    return n_split




def build_nc(phases="g0,r0,g1,r1,ln,crf"):
    phases = os.environ.get("NK_PHASES", phases).split(",")
    _install_bir_scrub()
    import concourse.bass as bass
    import concourse.mybir as mybir
    import concourse.tile as tile
    from concourse.bass import ds
    from concourse.masks import make_identity

    FP = mybir.dt.float32
    BF = mybir.dt.bfloat16
    AF = mybir.ActivationFunctionType

    nc = bass.Bass()

    def par(name, shape, out=False):
        return nc.declare_dram_parameter(name, list(shape), FP, isOutput=out)

    xT = par("xT", [E, ROWS])
    w0 = {d: par(f"w0{d}", [E, 4 * H]) for d in "fb"}
    w1 = {d: nc.declare_dram_parameter(f"w1{d}", [D, 4 * H], BF, isOutput=False) for d in "fb"}
    whh = {(l, d): nc.declare_dram_parameter(f"whh{l}{d}", [H, 4 * H], BF, isOutput=False) for l in (0, 1) for d in "fb"}
    bia = {(l, d): par(f"b{l}{d}", [1, 4 * H]) for l in (0, 1) for d in "fb"}
    gam = par("gam", [1, D])
    bet = par("bet", [1, D])
    woutT = par("woutT", [D, K])
    boutT = par("boutT", [K, 1])
    expTT = par("expTT", [K, K])
    stopR = par("stopR", [BC, K])
    alpha0 = par("alpha0", [BC, K])

    feats_out = par("feats_out", [K, ROWS], out=True)
    allsc_out = par("allsc_out", [BC, 1], out=True)

    wxs = {d: nc.dram_tensor(f"wxs_{d}", [ROWS, 4 * H], BF, kind="Internal")
           for d in "fb"}
    h1_d = nc.dram_tensor("h1_d", [ROWS, D], BF, kind="Internal")

    with tile.TileContext(nc) as tc:
        from contextlib import ExitStack
        with ExitStack() as es:
            const = es.enter_context(tc.tile_pool(name="const", bufs=1))
            big = es.enter_context(tc.tile_pool(name="big", bufs=1))

            ident = const.tile([128, 128], FP)
            make_identity(nc, ident)
            identB = const.tile([128, 128], BF)
            make_identity(nc, identB)
            ones1 = const.tile([1, 128], FP)
            nc.vector.memset(ones1, 1.0)
            eps_t = const.tile([128, 1], FP)
            nc.vector.memset(eps_t, 1e-5)
            lneps_t = const.tile([128, 1], FP)
            nc.vector.memset(lneps_t, 1e-38)

            featsT = big.tile([K, ROWS], FP)
            # rolling transposed-h buffers: 17 slots of 8 cols, per K-chunk
            hT = {(d, k): big.tile([128, 8 * (CHUNK + 1)], BF, name=f"hT{d}{k}", tag=f"hT{d}{k}")
                  for d in "fb" for k in (0, 1)}
            hTv = {key: t.rearrange("p (b s) -> p b s", s=CHUNK + 1)
                   for key, t in hT.items()}
            x1sb = {(d, k): big.tile([E, ROWS], BF, name=f"x1{d}{k}", tag=f"x1{d}{k}")
                    for d in "fb" for k in (0, 1)}
            x1v = {key: t.rearrange("p (b t) -> p b t", b=BC)
                   for key, t in x1sb.items()}
            c40 = big.tile([40, H], FP)

            # ---------------- input-projection GEMM ----------------
            def wx_gemm(layer, wih_tiles, bias_tiles, x1_resident):
                """wxs_d[rows, 4H] = x_rows @ Wih_d.T + (bih+bhh)"""
                with (
                    tc.tile_pool(name=f"g{layer}", bufs=2) as gp,
                    tc.tile_pool(name=f"g{layer}ps", bufs=2, space="PSUM") as gps,
                ):
                    if x1_resident:
                        # static loop; lhsT = resident bf16 layer-0 outputs
                        srcs = [x1sb["f", 0], x1sb["f", 1],
                                x1sb["b", 0], x1sb["b", 1]]
                        for mi in range(32):
                            for d in "fb":
                                ps = gps.tile([128, 4 * H], FP, name=f"ps{d}",
                                              tag=f"ps{d}")
                                for nh in range(2):
                                    nsl = slice(nh * 512, (nh + 1) * 512)
                                    for kc in range(4):
                                        nc.tensor.matmul(
                                            out=ps[:, nsl],
                                            lhsT=srcs[kc][:, mi * 128:(mi + 1) * 128],
                                            rhs=wih_tiles[d][kc][:, nsl],
                                            start=(kc == 0), stop=False)
                                    nc.tensor.matmul(
                                        out=ps[:, nsl], lhsT=ones1,
                                        rhs=bias_tiles[d][0:1, nsl],
                                        start=False, stop=True)
                                wxsb = gp.tile([128, 4 * H], BF, name=f"wxsb{d}",
                                               tag=f"wxsb{d}")
                                nc.scalar.copy(out=wxsb, in_=ps)
                                nc.sync.dma_start(
                                    out=wxs[d][mi * 128:(mi + 1) * 128, :],
                                    in_=wxsb)
                        return
                    with tc.For_i(0, 32) as mi:
                        xt = gp.tile([128, 128], FP, name="x1t0", tag="x1t0")
                        nc.sync.dma_start(out=xt, in_=xT[:, ds(mi * 128, 128)])
                        for d in "fb":
                            ps = gps.tile([128, 4 * H], FP, tag=f"ps{d}")
                            for nh in range(2):
                                nsl = slice(nh * 512, (nh + 1) * 512)
                                nc.tensor.matmul(
                                    out=ps[:, nsl], lhsT=xt,
                                    rhs=wih_tiles[d][0][:, nsl],
                                    start=True, stop=False)
                                nc.tensor.matmul(
                                    out=ps[:, nsl], lhsT=ones1,
                                    rhs=bias_tiles[d][0:1, nsl],
                                    start=False, stop=True)
                            wxsb = gp.tile([128, 4 * H], BF, name=f"wxsb{d}",
                                           tag=f"wxsb{d}")
                            nc.scalar.copy(out=wxsb, in_=ps)
                            nc.sync.dma_start(
                                out=wxs[d][ds(mi * 128, 128), :], in_=wxsb)

            # ---------------- recurrence ----------------
            def recurrence(layer, whh_tiles):
                for dk in hT.values():
                    nc.vector.memset(dk, 0.0)
                nc.vector.memset(c40, 0.0)
                with (
                    tc.tile_pool(name=f"r{layer}", bufs=2) as rp,
                    tc.tile_pool(name=f"r{layer}g", bufs=2, space="PSUM") as rg,
                    tc.tile_pool(name=f"r{layer}t", bufs=2, space="PSUM") as rt,
                ):
                    wxv = {d: wxs[d].rearrange("(b t) g -> b t g", b=BC)
                           for d in "fb"}
                    with tc.For_i(0, NCHUNK) as ci:
                        hc_f = rp.tile([8, CHUNK * H], BF, tag="hc_f")
                        hc_b = rp.tile([8, CHUNK * H], BF, tag="hc_b")
                        wxc_f = rp.tile([8, CHUNK * 4 * H], BF, tag="wxc_f")
                        nc.sync.dma_start(
                            out=wxc_f.rearrange("b (s g) -> b s g", g=4 * H),
                            in_=wxv["f"][:, ds(ci * CHUNK, CHUNK), :])
                        wxc_b = rp.tile([8, CHUNK * 4 * H], BF, tag="wxc_b")
                        nc.scalar.dma_start(
                            out=wxc_b.rearrange("b (s g) -> b s g", g=4 * H),
                            in_=wxv["b"][:, ds((T - CHUNK) - ci * CHUNK, CHUNK), :])
                        # carries: f slot[CHUNK]->slot0, b slot0->slot[CHUNK]
                        for k in (0, 1):
                            nc.scalar.copy(out=hTv["f", k][:, :, 0:1],
                                           in_=hTv["f", k][:, :, CHUNK:CHUNK + 1])
                            nc.scalar.copy(out=hTv["b", k][:, :, CHUNK:CHUNK + 1],
                                           in_=hTv["b", k][:, :, 0:1])
                        for s in range(CHUNK):
                            sb = CHUNK - 1 - s
                            gp = rg.tile([40, 4 * H], FP, tag="gp")
                            nc.scalar.copy(
                                out=gp[0:8, :],
                                in_=wxc_f[:, s * 1024:(s + 1) * 1024])
                            nc.vector.tensor_copy(
                                out=gp[32:40, :],
                                in_=wxc_b[:, sb * 1024:(sb + 1) * 1024])
                            for nh in range(2):
                                nsl = slice(nh * 512, (nh + 1) * 512)
                                for k in (0, 1):
                                    nc.tensor.matmul(
                                        out=gp[0:8, nsl],
                                        lhsT=hTv["f", k][:, :, s],
                                        rhs=whh_tiles["f"][k][:, nsl],
                                        start=False, stop=(k == 1),
                                        skip_group_check=True)
                                    nc.tensor.matmul(
                                        out=gp[32:40, nsl],
                                        lhsT=hTv["b", k][:, :, CHUNK - s],
                                        rhs=whh_tiles["b"][k][:, nsl],
                                        start=False, stop=(k == 1),
                                        skip_group_check=True)
                            sif = rp.tile([40, 2 * H], FP, tag="sif")
                            nc.scalar.activation(out=sif, in_=gp[:, 0:512],
                                                 func=AF.Sigmoid)
                            sg = rp.tile([40, H], FP, tag="sg")
                            nc.scalar.activation(out=sg, in_=gp[:, 512:768],
                                                 func=AF.Tanh)
                            so = rp.tile([40, H], FP, tag="so")
                            nc.scalar.activation(out=so, in_=gp[:, 768:1024],
                                                 func=AF.Sigmoid)
                            nc.gpsimd.tensor_mul(out=c40, in0=sif[:, 256:512], in1=c40)
                            igt = rp.tile([40, H], FP, tag="igt")
                            nc.gpsimd.tensor_mul(out=igt, in0=sif[:, 0:256], in1=sg)
                            nc.gpsimd.tensor_add(out=c40, in0=c40, in1=igt)
                            tch = rp.tile([40, H], FP, tag="tch")
                            nc.scalar.activation(out=tch, in_=c40, func=AF.Tanh)
                            nc.gpsimd.tensor_mul(
                                out=hc_f[:, s * H:(s + 1) * H],
                                in0=so[0:8, :], in1=tch[0:8, :])
                            nc.gpsimd.tensor_mul(
                                out=hc_b[:, sb * H:(sb + 1) * H],
                                in0=so[32:40, :], in1=tch[32:40, :])
                            for k in (0, 1):
                                ptf = rt.tile([128, 8], BF, tag="pt")
                                nc.tensor.transpose(
                                    out=ptf,
                                    in_=hc_f[:, s * H + k * 128: s * H + (k + 1) * 128],
                                    identity=identB[:8, :8])
                                nc.vector.tensor_copy(
                                    out=hTv["f", k][:, :, s + 1:s + 2], in_=ptf)
                                ptb = rt.tile([128, 8], BF, tag="pt")
                                nc.tensor.transpose(
                                    out=ptb,
                                    in_=hc_b[:, sb * H + k * 128: sb * H + (k + 1) * 128],
                                    identity=identB[:8, :8])
                                nc.vector.tensor_copy(
                                    out=hTv["b", k][:, :, sb:sb + 1], in_=ptb)
                        if layer == 1:
                            h1v = h1_d.rearrange("(b t) g -> b t g", b=BC)
                            hfv = hc_f.rearrange("b (s g) -> b s g", g=H)
                            hbv = hc_b.rearrange("b (s g) -> b s g", g=H)
                            nc.sync.dma_start(
                                out=h1v[:, ds(ci * CHUNK, CHUNK), 0:256], in_=hfv)
                            nc.scalar.dma_start(
                                out=h1v[:, ds((T - CHUNK) - ci * CHUNK, CHUNK), 256:512],
                                in_=hbv)
                        if layer == 0:
                            for k in (0, 1):
                                nc.vector.tensor_copy(
                                    out=x1v["f", k][:, :, ds(ci * CHUNK, CHUNK)],
                                    in_=hTv["f", k][:, :, 1:CHUNK + 1])
                                nc.vector.tensor_copy(
                                    out=x1v["b", k][:, :, ds((T - CHUNK) - ci * CHUNK, CHUNK)],
                                    in_=hTv["b", k][:, :, 0:CHUNK])

            # ---------------- layer 0 ----------------
            with tc.tile_pool(name="w0p", bufs=1) as w0p:
                w0t = {}
                b0t = {}
                whh0t = {}
                for d in "fb":
                    t0 = w0p.tile([E, 4 * H], FP, tag=f"w0{d}")
                    nc.sync.dma_start(out=t0, in_=w0[d][:, :])
                    w0t[d] = [t0]
                    bt = w0p.tile([1, 4 * H], FP, tag=f"b0{d}")
                    nc.sync.dma_start(out=bt, in_=bia[0, d][:, :])
                    b0t[d] = bt
                    whh0t[d] = []
                    for k in (0, 1):
                        wt = w0p.tile([128, 4 * H], BF, tag=f"whh0{d}{k}")
                        nc.sync.dma_start(out=wt, in_=whh[0, d][k * 128:(k + 1) * 128, :])
                        whh0t[d].append(wt)
                if "g0" in phases: wx_gemm(0, w0t, b0t, x1_resident=False)
                if "r0" in phases: recurrence(0, whh0t)

            # ---------------- layer 1 ----------------
            with tc.tile_pool(name="w1p", bufs=1) as w1p:
                w1t = {}
                b1t = {}
                whh1t = {}
                for d in "fb":
                    w1t[d] = []
                    for kc in range(4):
                        wt = w1p.tile([128, 4 * H], BF, tag=f"w1{d}{kc}")
                        nc.sync.dma_start(out=wt, in_=w1[d][kc * 128:(kc + 1) * 128, :])
                        w1t[d].append(wt)
                    bt = w1p.tile([1, 4 * H], FP, tag=f"b1{d}")
                    nc.sync.dma_start(out=bt, in_=bia[1, d][:, :])
                    b1t[d] = bt
                    whh1t[d] = []
                    for k in (0, 1):
                        wt = w1p.tile([128, 4 * H], BF, tag=f"whh1{d}{k}")
                        nc.sync.dma_start(out=wt, in_=whh[1, d][k * 128:(k + 1) * 128, :])
                        whh1t[d].append(wt)
                if "g1" in phases: wx_gemm(1, w1t, b1t, x1_resident=True)
                if "r1" in phases: recurrence(1, whh1t)

            # ---------------- LayerNorm + emission ----------------
            with (
                tc.tile_pool(name="lnp", bufs=3) as lp,
                tc.tile_pool(name="lnc", bufs=1) as lc,
                tc.tile_pool(name="lnps", bufs=2, space="PSUM") as lps,
            ):
                gamR = lc.tile([128, D], FP)
                ga = gam[:, :]
                nc.sync.dma_start(out=gamR, in_=bass.AP(
                    tensor=ga.tensor, offset=ga.offset, ap=[[0, 128]] + ga.ap[1:]))
                betR = lc.tile([128, D], FP)
                ba = bet[:, :]
                nc.sync.dma_start(out=betR, in_=bass.AP(
                    tensor=ba.tensor, offset=ba.offset, ap=[[0, 128]] + ba.ap[1:]))
                woutt = []
                for kc in range(4):
                    wt = lc.tile([128, K], FP, tag=f"wout{kc}")
                    nc.sync.dma_start(out=wt, in_=woutT[kc * 128:(kc + 1) * 128, :])
                    woutt.append(wt)
                boutt = lc.tile([K, 1], FP)
                nc.sync.dma_start(out=boutt, in_=boutT[:, :])

                with tc.For_i(0, 32 if "ln" in phases else 1) as ri:
                    xhb = lp.tile([128, D], BF, tag="xhb")
                    nc.sync.dma_start(out=xhb, in_=h1_d[ds(ri * 128, 128), :])
                    xh = lp.tile([128, D], FP, tag="xh")
                    nc.vector.tensor_copy(out=xh, in_=xhb)
                    stats = lp.tile([128, 6], FP, tag="stats")
                    nc.vector.bn_stats(out=stats, in_=xh)
                    mv = lp.tile([128, 2], FP, tag="mv")
                    nc.vector.bn_aggr(out=mv, in_=stats)
                    sd = lp.tile([128, 1], FP, tag="sd")
                    nc.scalar.activation(out=sd, in_=mv[:, 1:2], func=AF.Sqrt,
                                         bias=eps_t, scale=1.0)
                    rstd = lp.tile([128, 1], FP, tag="rstd")
                    nc.vector.reciprocal(out=rstd, in_=sd)
                    nc.vector.tensor_scalar(
                        out=xh, in0=xh, scalar1=mv[:, 0:1], scalar2=rstd,
                        op0=mybir.AluOpType.subtract, op1=mybir.AluOpType.mult)
                    nc.vector.tensor_mul(out=xh, in0=xh, in1=gamR)
                    nc.vector.tensor_add(out=xh, in0=xh, in1=betR)
                    fps = lps.tile([K, 128], FP, tag="fps")
                    for kc in range(4):
                        ntp = lps.tile([128, 128], FP, tag="ntp")
                        nc.tensor.transpose(out=ntp, in_=xh[:, kc * 128:(kc + 1) * 128],
                                            identity=ident)
                        ntk = lp.tile([128, 128], FP, tag="ntk")
                        nc.scalar.copy(out=ntk, in_=ntp)
                        nc.tensor.matmul(out=fps, lhsT=woutt[kc], rhs=ntk,
                                         start=(kc == 0), stop=(kc == 3))
                    nc.scalar.activation(out=featsT[:, ds(ri * 128, 128)], in_=fps,
                                         func=AF.Identity, bias=boutt, scale=1.0)

            # ---------------- CRF forward ----------------
            with (
                tc.tile_pool(name="crf", bufs=3) as cp,
                tc.tile_pool(name="crfc", bufs=1) as cc,
                tc.tile_pool(name="crfps", bufs=2, space="PSUM") as cps,
            ):
                expTTt = cc.tile([K, K], FP)
                nc.sync.dma_start(out=expTTt, in_=expTT[:, :])
                stopt = cc.tile([BC, K], FP)
                nc.sync.dma_start(out=stopt, in_=stopR[:, :])
                alpha = cc.tile([BC, K], FP)
                nc.sync.dma_start(out=alpha, in_=alpha0[:, :])

                CS = 8
                with tc.For_i(0, (T // CS) if "crf" in phases else 1) as ti:
                    for s in range(CS):
                        negm = cp.tile([BC, 1], FP, tag="negm")
                        nc.vector.reduce_max(out=negm, in_=alpha,
                                             axis=mybir.AxisListType.X, negate=True)
                        ea = cp.tile([BC, K], FP, tag="ea")
                        nc.scalar.activation(out=ea, in_=alpha, func=AF.Exp,
                                             bias=negm, scale=1.0)
                        eap = cps.tile([K, BC], FP, tag="eap")
                        nc.tensor.transpose(out=eap, in_=ea, identity=ident[:BC, :BC])
                        eas = cp.tile([K, BC], FP, tag="eas")
                        nc.scalar.copy(out=eas, in_=eap)
                        smp = cps.tile([K, BC], FP, tag="smp")
                        nc.tensor.matmul(out=smp, lhsT=expTTt, rhs=eas,
                                         start=True, stop=True)
                        la = cp.tile([K, BC], FP, tag="la")
                        nc.scalar.activation(out=la, in_=smp, func=AF.Ln,
                                             bias=lneps_t[:K], scale=1.0)
                        u = cp.tile([K, BC], FP, tag="u")
                        fv = featsT.rearrange("k (b t) -> k b t", b=BC)
                        eng = nc.vector if s % 2 == 0 else nc.gpsimd
                        eng.tensor_add(out=u, in0=la,
                                       in1=fv[:, :, ds(ti * CS + s, 1)])
                        up = cps.tile([BC, K], FP, tag="up")
                        nc.tensor.transpose(out=up, in_=u, identity=ident[:K, :K])
                        m = cp.tile([BC, 1], FP, tag="m")
                        nc.vector.tensor_scalar_mul(out=m, in0=negm, scalar1=-1.0)
                        nc.scalar.activation(out=alpha, in_=up, func=AF.Identity,
                                             bias=m, scale=1.0)
                u2 = cp.tile([BC, K], FP, tag="u2")
                nc.vector.tensor_add(out=u2, in0=alpha, in1=stopt)
                negm2 = cp.tile([BC, 1], FP, tag="negm2")
                nc.vector.reduce_max(out=negm2, in_=u2,
                                     axis=mybir.AxisListType.X, negate=True)
                es2 = cp.tile([BC, K], FP, tag="es2")
                sm2 = cp.tile([BC, 1], FP, tag="sm2")
                nc.scalar.activation(out=es2, in_=u2, func=AF.Exp,
                                     bias=negm2, scale=1.0, accum_out=sm2)
                ln2 = cp.tile([BC, 1], FP, tag="ln2")
                nc.scalar.activation(out=ln2, in_=sm2, func=AF.Ln)
                allsc = cp.tile([BC, 1], FP, tag="allsc")
                nc.vector.tensor_sub(out=allsc, in0=ln2, in1=negm2)
                nc.sync.dma_start(out=allsc_out[:, :], in_=allsc)
                nc.sync.dma_start(out=feats_out[:, :], in_=featsT)

    return nc


def _prep_inputs(inp):
    """Host-side prep: embedding gather + transposes. Returns (per_core, shared)."""
    x = inp["embed"][inp["sentence"]]  # [B, T, E] f32
    per_core = []
    for c in range(NCORES):
        xc = x[c * BC:(c + 1) * BC]  # [8, T, E]
        xTc = np.ascontiguousarray(xc.transpose(2, 0, 1).reshape(E, ROWS))
        per_core.append(xTc.astype(np.float32, copy=False))

    sh = {}
    for l, key in ((0, "l0"), (1, "l1")):
        for d in "fb":
            sh[f"w{l}{d}"] = np.ascontiguousarray(inp[f"Wih_{key}{d}"].T)
            sh[f"whh{l}{d}"] = np.ascontiguousarray(inp[f"Whh_{key}{d}"].T)
            sh[f"b{l}{d}"] = (inp[f"bih_{key}{d}"] + inp[f"bhh_{key}{d}"])[None, :]
    sh["gam"] = inp["ln_gamma"][None, :]
    sh["bet"] = inp["ln_beta"][None, :]
    sh["woutT"] = np.ascontiguousarray(inp["Wout"].T)
    sh["boutT"] = inp["bout"][:, None]
    trans = inp["transitions"]
    sh["expTT"] = np.ascontiguousarray(np.exp(trans).T)
    sh["stopR"] = np.repeat(trans[STOP][None, :], BC, axis=0)
    a0 = np.full((BC, K), NEG, np.float32)
    a0[:, START] = 0.0
    sh["alpha0"] = a0
    import ml_dtypes
    sh = {k: np.ascontiguousarray(v.astype(np.float32, copy=False))
          for k, v in sh.items()}
    for key in ("w1f", "w1b", "whh0f", "whh0b", "whh1f", "whh1b"):
        sh[key] = sh[key].astype(ml_dtypes.bfloat16)
    return per_core, sh


def _real_path(feats, tags, trans):
    # feats: [B, T, K]
    b = feats.shape[0]
    tf = np.concatenate([np.full((b, 1), START, tags.dtype), tags], axis=1)
    trans_sc = trans[tf[:, 1:], tf[:, :-1]].sum(axis=1, dtype=np.float32)
    emit_sc = np.take_along_axis(
        feats, tags[:, :, None].astype(np.int64), axis=2)[..., 0].sum(
        axis=1, dtype=np.float32)
    stop_sc = trans[STOP, tags[:, -1]]
    return trans_sc + emit_sc + stop_sc



_runner = None


class _Runner:
    """Persistent jitted SPMD executor for the bass kernel (replicates the
    multi-core path of bass2jax.run_bass_via_pjrt, but keeps the jitted
    callable and device-resident inputs so repeat calls measure execution,
    not retracing/transfers)."""

    def __init__(self, nc):
        import jax
        from jax.experimental.shard_map import shard_map
        from jax.sharding import Mesh, NamedSharding, PartitionSpec
        from concourse import bass2jax, mybir

        bass2jax.install_neuronx_cc_hook()
        self.jax = jax
        self.nc = nc
        partition_name = (nc.partition_id_tensor.name
                          if nc.partition_id_tensor else None)
        in_names, out_names, out_avals, zero_outs = [], [], [], []
        for alloc in nc.m.functions[0].allocations:
            if not isinstance(alloc, mybir.MemoryLocationSet):
                continue
            name = alloc.memorylocations[0].name
            if alloc.kind == "ExternalInput":
                if name != partition_name:
                    in_names.append(name)
            elif alloc.kind == "ExternalOutput":
                out_names.append(name)
                shape = tuple(alloc.tensor_shape)
                dtype = mybir.dt.np(alloc.dtype)
                out_avals.append(jax.core.ShapedArray(shape, dtype))
                zero_outs.append(np.zeros(shape, dtype))
        self.n_params = len(in_names)
        self.in_names = list(in_names)
        self.out_names = out_names
        self.out_avals = out_avals
        self.zero_outs = zero_outs
        all_in_names = in_names + out_names
        if partition_name is not None:
            all_in_names.append(partition_name)

        def _body(*args):
            operands = list(args)
            if partition_name is not None:
                operands.append(bass2jax.partition_id_tensor())
            outs = bass2jax._bass_exec_p.bind(
                *operands,
                out_avals=tuple(out_avals),
                in_names=tuple(all_in_names),
                out_names=tuple(out_names),
                lowering_input_output_aliases=(),
                sim_require_finite=True,
                sim_require_nnan=True,
                nc=nc,
            )
            return tuple(outs)

        devices = jax.devices()[:NCORES]
        self.mesh = Mesh(np.asarray(devices), ("core",))
        n_outs = len(out_avals)
        in_specs = (PartitionSpec("core"),) * (self.n_params + n_outs)
        out_specs = (PartitionSpec("core"),) * n_outs
        donate = tuple(range(self.n_params, self.n_params + n_outs))
        self.sharding = NamedSharding(self.mesh, PartitionSpec("core"))
        self.fn = jax.jit(
            shard_map(_body, mesh=self.mesh, in_specs=in_specs,
                      out_specs=out_specs, check_rep=False),
            donate_argnums=donate, keep_unused=True)
        self.placed = None
        self._zpool = []

    def place_inputs(self, in_maps):
        concat = [
            np.concatenate([np.asarray(in_maps[c][n]) for c in range(NCORES)],
                           axis=0)
            for n in self.in_names
        ]
        self.placed = [self.jax.device_put(a, self.sharding) for a in concat]
        self.jax.block_until_ready(self.placed)

    def _one_zero_set(self):
        zs = [self.jax.device_put(
                  np.zeros((NCORES * z.shape[0], *z.shape[1:]), z.dtype),
                  self.sharding)
              for z in self.zero_outs]
        self.jax.block_until_ready(zs)
        return zs

    def prewarm_zeros(self, n):
        """Pre-place n donated zero-output sets on device so timed
        executions contain no host->device transfers."""
        while len(self._zpool) < n:
            self._zpool.append(self._one_zero_set())

    def execute(self):
        """Run on pre-placed inputs; returns device arrays (no D2H)."""
        zs = self._zpool.pop() if self._zpool else self._one_zero_set()
        outs = self.fn(*self.placed, *zs)
        self.jax.block_until_ready(outs)
        return outs

    def execute_pipelined(self, n):
        """Launch n kernel executions back-to-back without host blocking
        between them (jax async dispatch), block once at the end. Returns
        total seconds. Amortizes the per-call axon RPC/dispatch floor so
        per-iteration time approaches true device execution time."""
        import time as _time
        self.prewarm_zeros(n)
        zsets = [self._zpool.pop() for _ in range(n)]
        t0 = _time.perf_counter()
        outs = [self.fn(*self.placed, *zs) for zs in zsets]
        self.jax.block_until_ready(outs)
        return _time.perf_counter() - t0

    def run(self, in_maps):
        self.place_inputs(in_maps)
        outs = self.execute()
        results = []
        for c in range(NCORES):
            results.append({
                name: np.asarray(outs[i]).reshape(
                    NCORES, *self.out_avals[i].shape)[c]
                for i, name in enumerate(self.out_names)
            })
        return results


def get_runner():
    global _runner, _nc_cache
    if _runner is None:
        if _nc_cache is None:
            nc = build_nc()
            _split_waits(nc)
            _nc_cache = nc
        _runner = _Runner(_nc_cache)
    return _runner


def kernel(**inputs) -> np.ndarray:
    inp = {k: np.asarray(v) for k, v in inputs.items()}
    per_core, sh = _prep_inputs(inp)

    global _nc_cache
    if _nc_cache is None:
        nc = build_nc()
        _split_waits(nc)
        _nc_cache = nc
    nc = _nc_cache

    in_maps = [dict(sh, xT=per_core[c]) for c in range(NCORES)]
    runner = get_runner()
    results = runner.run(in_maps)

    all_sc = np.concatenate(
        [results[c]["allsc_out"][:, 0] for c in range(NCORES)])  # [B]
    feats = np.concatenate(
        [results[c]["feats_out"].reshape(K, BC, T).transpose(1, 2, 0)
         for c in range(NCORES)], axis=0)  # [B, T, K]
    real_sc = _real_path(feats, inp["tags"], inp["transitions"])
    loss = (all_sc - real_sc).mean(dtype=np.float32)
    return np.asarray(loss, dtype=np.float32)


def kernel(**inputs) -> np.ndarray:
    inp = {k: np.asarray(v) for k, v in inputs.items()}
    per_core, sh = _prep_inputs(inp)

    in_maps = [dict(sh, xT=per_core[c]) for c in range(NCORES)]
    runner = get_runner()
    results = runner.run(in_maps)

    all_sc = np.concatenate(
        [results[c]["allsc_out"][:, 0] for c in range(NCORES)])  # [B]
    feats = np.concatenate(
        [results[c]["feats_out"].reshape(K, BC, T).transpose(1, 2, 0)
         for c in range(NCORES)], axis=0)  # [B, T, K]
    real_sc = _real_path(feats, inp["tags"], inp["transitions"])
    loss = (all_sc - real_sc).mean(dtype=np.float32)
    return np.asarray(loss, dtype=np.float32)


# revision 12
# speedup vs baseline: 1.1054x; 1.1054x over previous
"""BiLSTM-CRF loss on 8 Trainium2 NeuronCores — full device implementation.

Per core (8 sequences): 2-layer BiLSTM (input projections as GEMMs + fused
f/b recurrence), LayerNorm + emission GEMM, CRF forward pass via the
exp-transition matmul trick. Host does only: embedding gather/transpose,
weight pre-transposes, CRF real-path score (gather-heavy), final mean.

Layout notes:
- "rows" are t-major: row = t*8 + b_local (t in [0,512), b_local in [0,8)).
- Recurrence tiles hold fwd dir in partitions 0:8 and bwd dir in 32:40
  (compute-engine partition bases must be 32-aligned; rows 8:32 are junk).
- Backward direction runs on reversed time internally; all reversals are
  static index arithmetic, no reversed DMAs.
"""

import os
import sys

os.environ.setdefault("JAX_COMPILATION_CACHE_DIR", "/root/.cache/jax_bass_cache")
os.environ.setdefault("JAX_PERSISTENT_CACHE_MIN_ENTRY_SIZE_BYTES", "-1")
os.environ.setdefault("JAX_PERSISTENT_CACHE_MIN_COMPILE_TIME_SECS", "0")

import numpy as np

for _p in ("/opt/trn_rl_repo",):
    if _p not in sys.path:
        sys.path.append(_p)

def _install_bir_scrub():
    """Make serialized BIR independent of this file's location: debug-info
    filenames otherwise change the compiler cache key per directory."""
    import concourse.bass as bass
    if getattr(bass.Bass, "_bir_scrubbed", False):
        return
    orig = bass.Bass.to_json_bytes
    here = os.path.abspath(__file__).encode()

    def scrubbed(self):
        return orig(self).replace(here, b"/k.py")

    bass.Bass.to_json_bytes = scrubbed
    bass.Bass._bir_scrubbed = True


B, T, V, E, D, K = 64, 512, 8000, 128, 512, 35
H = D // 2
START, STOP = 33, 34
NEG = -10000.0
NCORES = 8
BC = B // NCORES  # 8
ROWS = BC * T  # 4096
CHUNK = 8
NCHUNK = T // CHUNK  # 64

_nc_cache = None


def _split_waits(nc):
    """Hoist extra sync waits onto NoOps: this walrus build rejects
    instructions carrying more than one sync wait. Engines execute their
    stream in order, so a preceding same-engine NoOp wait is equivalent."""
    from concourse import mybir
    n_split = 0
    for func in nc.m.functions:
        for block in func.blocks:
            out = []
            changed = False
            for inst in block.instructions:
                si = inst.sync_info
                waits = list(si.on_wait) if si is not None else []
                if len(waits) > 1:
                    changed = True
                    for w in waits[:-1]:
                        n_split += 1
                        out.append(mybir.InstNoOp(
                            name=f"{inst.name}-wsplit{n_split}",
                            engine=inst.engine, ins=[], outs=[],
                            sync_info=mybir.SyncInfo(on_wait=[w], on_update=[])))
                    si.on_wait = waits[-1:]
                out.append(inst)
            if changed:
                block.instructions = out
    # Strip source-path debug info so the serialized BIR (and thus the
    # compiler cache key) is independent of where kernel.py lives.
    for func in nc.m.functions:
        for block in func.blocks:
            for inst in block.instructions:
                try:
                    inst.debug = None
                except Exception:
                    pass
                try:
                    inst.bass_addl_debug = None
                except Exception:
                    pass
    return n_split




def build_nc(phases="g0,r0,g1,r1,ln,crf"):
    phases = os.environ.get("NK_PHASES", phases).split(",")
    _install_bir_scrub()
    import concourse.bass as bass
    import concourse.mybir as mybir
    import concourse.tile as tile
    from concourse.bass import ds
    from concourse.masks import make_identity

    FP = mybir.dt.float32
    BF = mybir.dt.bfloat16
    AF = mybir.ActivationFunctionType

    nc = bass.Bass()

    def par(name, shape, out=False):
        return nc.declare_dram_parameter(name, list(shape), FP, isOutput=out)

    xT = par("xT", [E, ROWS])
    w0 = {d: par(f"w0{d}", [E, 4 * H]) for d in "fb"}
    w1 = {d: nc.declare_dram_parameter(f"w1{d}", [D, 4 * H], BF, isOutput=False) for d in "fb"}
    whh = {(l, d): nc.declare_dram_parameter(f"whh{l}{d}", [H, 4 * H], BF, isOutput=False) for l in (0, 1) for d in "fb"}
    bia = {(l, d): par(f"b{l}{d}", [1, 4 * H]) for l in (0, 1) for d in "fb"}
    gam = par("gam", [1, D])
    bet = par("bet", [1, D])
    woutT = par("woutT", [D, K])
    boutT = par("boutT", [K, 1])
    expTT = par("expTT", [K, K])
    stopR = par("stopR", [BC, K])
    alpha0 = par("alpha0", [BC, K])

    feats_out = par("feats_out", [K, ROWS], out=True)
    allsc_out = par("allsc_out", [BC, 1], out=True)

    wxs = {d: nc.dram_tensor(f"wxs_{d}", [ROWS, 4 * H], BF, kind="Internal")
           for d in "fb"}
    h1_d = nc.dram_tensor("h1_d", [ROWS, D], BF, kind="Internal")

    with tile.TileContext(nc) as tc:
        from contextlib import ExitStack
        with ExitStack() as es:
            const = es.enter_context(tc.tile_pool(name="const", bufs=1))
            big = es.enter_context(tc.tile_pool(name="big", bufs=1))

            ident = const.tile([128, 128], FP)
            make_identity(nc, ident)
            identB = const.tile([128, 128], BF)
            make_identity(nc, identB)
            ones1 = const.tile([1, 128], FP)
            nc.vector.memset(ones1, 1.0)
            eps_t = const.tile([128, 1], FP)
            nc.vector.memset(eps_t, 1e-5)
            lneps_t = const.tile([128, 1], FP)
            nc.vector.memset(lneps_t, 1e-38)

            featsT = big.tile([K, ROWS], FP)
            # rolling transposed-h buffers: 17 slots of 8 cols, per K-chunk
            hT = {(d, k): big.tile([128, 8 * (CHUNK + 1)], BF, name=f"hT{d}{k}", tag=f"hT{d}{k}")
                  for d in "fb" for k in (0, 1)}
            hTv = {key: t.rearrange("p (b s) -> p b s", s=CHUNK + 1)
                   for key, t in hT.items()}
            x1sb = {(d, k): big.tile([E, ROWS], BF, name=f"x1{d}{k}", tag=f"x1{d}{k}")
                    for d in "fb" for k in (0, 1)}
            x1v = {key: t.rearrange("p (b t) -> p b t", b=BC)
                   for key, t in x1sb.items()}
            c40 = big.tile([40, H], FP)

            # ---------------- input-projection GEMM ----------------
            def wx_gemm(layer, wih_tiles, bias_tiles, x1_resident):
                """wxs_d[rows, 4H] = x_rows @ Wih_d.T + (bih+bhh)"""
                with (
                    tc.tile_pool(name=f"g{layer}", bufs=2) as gp,
                    tc.tile_pool(name=f"g{layer}ps", bufs=2, space="PSUM") as gps,
                ):
                    if x1_resident:
                        # static loop; lhsT = resident bf16 layer-0 outputs
                        srcs = [x1sb["f", 0], x1sb["f", 1],
                                x1sb["b", 0], x1sb["b", 1]]
                        for mi in range(32):
                            for d in "fb":
                                ps = gps.tile([128, 4 * H], FP, name=f"ps{d}",
                                              tag=f"ps{d}")
                                for nh in range(2):
                                    nsl = slice(nh * 512, (nh + 1) * 512)
                                    for kc in range(4):
                                        nc.tensor.matmul(
                                            out=ps[:, nsl],
                                            lhsT=srcs[kc][:, mi * 128:(mi + 1) * 128],
                                            rhs=wih_tiles[d][kc][:, nsl],
                                            start=(kc == 0), stop=False)
                                    nc.tensor.matmul(
                                        out=ps[:, nsl], lhsT=ones1,
                                        rhs=bias_tiles[d][0:1, nsl],
                                        start=False, stop=True)
                                wxsb = gp.tile([128, 4 * H], BF, name=f"wxsb{d}",
                                               tag=f"wxsb{d}")
                                nc.scalar.copy(out=wxsb, in_=ps)
                                nc.sync.dma_start(
                                    out=wxs[d][mi * 128:(mi + 1) * 128, :],
                                    in_=wxsb)
                        return
                    with tc.For_i(0, 32) as mi:
                        xt = gp.tile([128, 128], FP, name="x1t0", tag="x1t0")
                        nc.sync.dma_start(out=xt, in_=xT[:, ds(mi * 128, 128)])
                        for d in "fb":
                            ps = gps.tile([128, 4 * H], FP, tag=f"ps{d}")
                            for nh in range(2):
                                nsl = slice(nh * 512, (nh + 1) * 512)
                                nc.tensor.matmul(
                                    out=ps[:, nsl], lhsT=xt,
                                    rhs=wih_tiles[d][0][:, nsl],
                                    start=True, stop=False)
                                nc.tensor.matmul(
                                    out=ps[:, nsl], lhsT=ones1,
                                    rhs=bias_tiles[d][0:1, nsl],
                                    start=False, stop=True)
                            wxsb = gp.tile([128, 4 * H], BF, name=f"wxsb{d}",
                                           tag=f"wxsb{d}")
                            nc.scalar.copy(out=wxsb, in_=ps)
                            nc.sync.dma_start(
                                out=wxs[d][ds(mi * 128, 128), :], in_=wxsb)

            # ---------------- recurrence ----------------
            def recurrence(layer, whh_tiles):
                for dk in hT.values():
                    nc.vector.memset(dk, 0.0)
                nc.vector.memset(c40, 0.0)
                with (
                    tc.tile_pool(name=f"r{layer}", bufs=2) as rp,
                    tc.tile_pool(name=f"r{layer}g", bufs=1, space="PSUM") as rg,
                    tc.tile_pool(name=f"r{layer}t", bufs=2, space="PSUM") as rt,
                ):
                    wxv = {d: wxs[d].rearrange("(b t) g -> b t g", b=BC)
                           for d in "fb"}
                    with tc.For_i(0, NCHUNK) as ci:
                        hc_f = rp.tile([8, CHUNK * H], BF, tag="hc_f")
                        hc_b = rp.tile([8, CHUNK * H], BF, tag="hc_b")
                        wxc_f = rp.tile([8, CHUNK * 4 * H], BF, tag="wxc_f")
                        nc.sync.dma_start(
                            out=wxc_f.rearrange("b (s g) -> b s g", g=4 * H),
                            in_=wxv["f"][:, ds(ci * CHUNK, CHUNK), :])
                        wxc_b = rp.tile([8, CHUNK * 4 * H], BF, tag="wxc_b")
                        nc.scalar.dma_start(
                            out=wxc_b.rearrange("b (s g) -> b s g", g=4 * H),
                            in_=wxv["b"][:, ds((T - CHUNK) - ci * CHUNK, CHUNK), :])
                        # carries: f slot[CHUNK]->slot0, b slot0->slot[CHUNK]
                        for k in (0, 1):
                            nc.scalar.copy(out=hTv["f", k][:, :, 0:1],
                                           in_=hTv["f", k][:, :, CHUNK:CHUNK + 1])
                            nc.scalar.copy(out=hTv["b", k][:, :, CHUNK:CHUNK + 1],
                                           in_=hTv["b", k][:, :, 0:1])
                        for s in range(CHUNK):
                            sb = CHUNK - 1 - s
                            gp = rg.tile([40, 4 * H], FP, tag="gp")
                            nc.scalar.copy(
                                out=gp[0:8, :],
                                in_=wxc_f[:, s * 1024:(s + 1) * 1024])
                            nc.vector.tensor_copy(
                                out=gp[32:40, :],
                                in_=wxc_b[:, sb * 1024:(sb + 1) * 1024])
                            for nh in range(2):
                                nsl = slice(nh * 512, (nh + 1) * 512)
                                for k in (0, 1):
                                    nc.tensor.matmul(
                                        out=gp[0:8, nsl],
                                        lhsT=hTv["f", k][:, :, s],
                                        rhs=whh_tiles["f"][k][:, nsl],
                                        start=False, stop=(k == 1),
                                        skip_group_check=True)
                                    nc.tensor.matmul(
                                        out=gp[32:40, nsl],
                                        lhsT=hTv["b", k][:, :, CHUNK - s],
                                        rhs=whh_tiles["b"][k][:, nsl],
                                        start=False, stop=(k == 1),
                                        skip_group_check=True)
                            sif = rp.tile([40, 2 * H], FP, tag="sif")
                            nc.scalar.activation(out=sif, in_=gp[:, 0:512],
                                                 func=AF.Sigmoid)
                            sg = rp.tile([40, H], FP, tag="sg")
                            nc.scalar.activation(out=sg, in_=gp[:, 512:768],
                                                 func=AF.Tanh)
                            so = rp.tile([40, H], FP, tag="so")
                            nc.scalar.activation(out=so, in_=gp[:, 768:1024],
                                                 func=AF.Sigmoid)
                            nc.gpsimd.tensor_mul(out=c40, in0=sif[:, 256:512], in1=c40)
                            igt = rp.tile([40, H], FP, tag="igt")
                            nc.gpsimd.tensor_mul(out=igt, in0=sif[:, 0:256], in1=sg)
                            nc.gpsimd.tensor_add(out=c40, in0=c40, in1=igt)
                            tch = rp.tile([40, H], FP, tag="tch")
                            nc.scalar.activation(out=tch, in_=c40, func=AF.Tanh)
                            nc.gpsimd.tensor_mul(
                                out=hc_f[:, s * H:(s + 1) * H],
                                in0=so[0:8, :], in1=tch[0:8, :])
                            nc.gpsimd.tensor_mul(
                                out=hc_b[:, sb * H:(sb + 1) * H],
                                in0=so[32:40, :], in1=tch[32:40, :])
                            for k in (0, 1):
                                ptf = rt.tile([128, 8], BF, tag="pt")
                                nc.tensor.transpose(
                                    out=ptf,
                                    in_=hc_f[:, s * H + k * 128: s * H + (k + 1) * 128],
                                    identity=identB[:8, :8])
                                nc.vector.tensor_copy(
                                    out=hTv["f", k][:, :, s + 1:s + 2], in_=ptf)
                                ptb = rt.tile([128, 8], BF, tag="pt")
                                nc.tensor.transpose(
                                    out=ptb,
                                    in_=hc_b[:, sb * H + k * 128: sb * H + (k + 1) * 128],
                                    identity=identB[:8, :8])
                                nc.vector.tensor_copy(
                                    out=hTv["b", k][:, :, sb:sb + 1], in_=ptb)
                        if layer == 1:
                            h1v = h1_d.rearrange("(b t) g -> b t g", b=BC)
                            hfv = hc_f.rearrange("b (s g) -> b s g", g=H)
                            hbv = hc_b.rearrange("b (s g) -> b s g", g=H)
                            nc.sync.dma_start(
                                out=h1v[:, ds(ci * CHUNK, CHUNK), 0:256], in_=hfv)
                            nc.scalar.dma_start(
                                out=h1v[:, ds((T - CHUNK) - ci * CHUNK, CHUNK), 256:512],
                                in_=hbv)
                        if layer == 0:
                            for k in (0, 1):
                                nc.vector.tensor_copy(
                                    out=x1v["f", k][:, :, ds(ci * CHUNK, CHUNK)],
                                    in_=hTv["f", k][:, :, 1:CHUNK + 1])
                                nc.vector.tensor_copy(
                                    out=x1v["b", k][:, :, ds((T - CHUNK) - ci * CHUNK, CHUNK)],
                                    in_=hTv["b", k][:, :, 0:CHUNK])

            # ---------------- layer 0 ----------------
            with tc.tile_pool(name="w0p", bufs=1) as w0p:
                w0t = {}
                b0t = {}
                whh0t = {}
                for d in "fb":
                    t0 = w0p.tile([E, 4 * H], FP, tag=f"w0{d}")
                    nc.sync.dma_start(out=t0, in_=w0[d][:, :])
                    w0t[d] = [t0]
                    bt = w0p.tile([1, 4 * H], FP, tag=f"b0{d}")
                    nc.sync.dma_start(out=bt, in_=bia[0, d][:, :])
                    b0t[d] = bt
                    whh0t[d] = []
                    for k in (0, 1):
                        wt = w0p.tile([128, 4 * H], BF, tag=f"whh0{d}{k}")
                        nc.sync.dma_start(out=wt, in_=whh[0, d][k * 128:(k + 1) * 128, :])
                        whh0t[d].append(wt)
                if "g0" in phases: wx_gemm(0, w0t, b0t, x1_resident=False)
                if "r0" in phases: recurrence(0, whh0t)

            # ---------------- layer 1 ----------------
            with tc.tile_pool(name="w1p", bufs=1) as w1p:
                w1t = {}
                b1t = {}
                whh1t = {}
                for d in "fb":
                    w1t[d] = []
                    for kc in range(4):
                        wt = w1p.tile([128, 4 * H], BF, tag=f"w1{d}{kc}")
                        nc.sync.dma_start(out=wt, in_=w1[d][kc * 128:(kc + 1) * 128, :])
                        w1t[d].append(wt)
                    bt = w1p.tile([1, 4 * H], FP, tag=f"b1{d}")
                    nc.sync.dma_start(out=bt, in_=bia[1, d][:, :])
                    b1t[d] = bt
                    whh1t[d] = []
                    for k in (0, 1):
                        wt = w1p.tile([128, 4 * H], BF, tag=f"whh1{d}{k}")
                        nc.sync.dma_start(out=wt, in_=whh[1, d][k * 128:(k + 1) * 128, :])
                        whh1t[d].append(wt)
                if "g1" in phases: wx_gemm(1, w1t, b1t, x1_resident=True)
                if "r1" in phases: recurrence(1, whh1t)

            # ---------------- LayerNorm + emission ----------------
            with (
                tc.tile_pool(name="lnp", bufs=3) as lp,
                tc.tile_pool(name="lnc", bufs=1) as lc,
                tc.tile_pool(name="lnps", bufs=2, space="PSUM") as lps,
            ):
                gamR = lc.tile([128, D], FP)
                ga = gam[:, :]
                nc.sync.dma_start(out=gamR, in_=bass.AP(
                    tensor=ga.tensor, offset=ga.offset, ap=[[0, 128]] + ga.ap[1:]))
                betR = lc.tile([128, D], FP)
                ba = bet[:, :]
                nc.sync.dma_start(out=betR, in_=bass.AP(
                    tensor=ba.tensor, offset=ba.offset, ap=[[0, 128]] + ba.ap[1:]))
                woutt = []
                for kc in range(4):
                    wt = lc.tile([128, K], FP, tag=f"wout{kc}")
                    nc.sync.dma_start(out=wt, in_=woutT[kc * 128:(kc + 1) * 128, :])
                    woutt.append(wt)
                boutt = lc.tile([K, 1], FP)
                nc.sync.dma_start(out=boutt, in_=boutT[:, :])

                with tc.For_i(0, 32 if "ln" in phases else 1) as ri:
                    xhb = lp.tile([128, D], BF, tag="xhb")
                    nc.sync.dma_start(out=xhb, in_=h1_d[ds(ri * 128, 128), :])
                    xh = lp.tile([128, D], FP, tag="xh")
                    nc.vector.tensor_copy(out=xh, in_=xhb)
                    stats = lp.tile([128, 6], FP, tag="stats")
                    nc.vector.bn_stats(out=stats, in_=xh)
                    mv = lp.tile([128, 2], FP, tag="mv")
                    nc.vector.bn_aggr(out=mv, in_=stats)
                    sd = lp.tile([128, 1], FP, tag="sd")
                    nc.scalar.activation(out=sd, in_=mv[:, 1:2], func=AF.Sqrt,
                                         bias=eps_t, scale=1.0)
                    rstd = lp.tile([128, 1], FP, tag="rstd")
                    nc.vector.reciprocal(out=rstd, in_=sd)
                    nc.vector.tensor_scalar(
                        out=xh, in0=xh, scalar1=mv[:, 0:1], scalar2=rstd,
                        op0=mybir.AluOpType.subtract, op1=mybir.AluOpType.mult)
                    nc.vector.tensor_mul(out=xh, in0=xh, in1=gamR)
                    nc.vector.tensor_add(out=xh, in0=xh, in1=betR)
                    fps = lps.tile([K, 128], FP, tag="fps")
                    for kc in range(4):
                        ntp = lps.tile([128, 128], FP, tag="ntp")
                        nc.tensor.transpose(out=ntp, in_=xh[:, kc * 128:(kc + 1) * 128],
                                            identity=ident)
                        ntk = lp.tile([128, 128], FP, tag="ntk")
                        nc.scalar.copy(out=ntk, in_=ntp)
                        nc.tensor.matmul(out=fps, lhsT=woutt[kc], rhs=ntk,
                                         start=(kc == 0), stop=(kc == 3))
                    nc.scalar.activation(out=featsT[:, ds(ri * 128, 128)], in_=fps,
                                         func=AF.Identity, bias=boutt, scale=1.0)

            # ---------------- CRF forward ----------------
            with (
                tc.tile_pool(name="crf", bufs=3) as cp,
                tc.tile_pool(name="crfc", bufs=1) as cc,
                tc.tile_pool(name="crfps", bufs=2, space="PSUM") as cps,
            ):
                expTTt = cc.tile([K, K], FP)
                nc.sync.dma_start(out=expTTt, in_=expTT[:, :])
                stopt = cc.tile([BC, K], FP)
                nc.sync.dma_start(out=stopt, in_=stopR[:, :])
                alpha = cc.tile([BC, K], FP)
                nc.sync.dma_start(out=alpha, in_=alpha0[:, :])

                CS = 8
                with tc.For_i(0, (T // CS) if "crf" in phases else 1) as ti:
                    for s in range(CS):
                        negm = cp.tile([BC, 1], FP, tag="negm")
                        nc.vector.reduce_max(out=negm, in_=alpha,
                                             axis=mybir.AxisListType.X, negate=True)
                        ea = cp.tile([BC, K], FP, tag="ea")
                        nc.scalar.activation(out=ea, in_=alpha, func=AF.Exp,
                                             bias=negm, scale=1.0)
                        eap = cps.tile([K, BC], FP, tag="eap")
                        nc.tensor.transpose(out=eap, in_=ea, identity=ident[:BC, :BC])
                        eas = cp.tile([K, BC], FP, tag="eas")
                        nc.scalar.copy(out=eas, in_=eap)
                        smp = cps.tile([K, BC], FP, tag="smp")
                        nc.tensor.matmul(out=smp, lhsT=expTTt, rhs=eas,
                                         start=True, stop=True)
                        la = cp.tile([K, BC], FP, tag="la")
                        nc.scalar.activation(out=la, in_=smp, func=AF.Ln,
                                             bias=lneps_t[:K], scale=1.0)
                        u = cp.tile([K, BC], FP, tag="u")
                        fv = featsT.rearrange("k (b t) -> k b t", b=BC)
                        eng = nc.vector if s % 2 == 0 else nc.gpsimd
                        eng.tensor_add(out=u, in0=la,
                                       in1=fv[:, :, ds(ti * CS + s, 1)])
                        up = cps.tile([BC, K], FP, tag="up")
                        nc.tensor.transpose(out=up, in_=u, identity=ident[:K, :K])
                        m = cp.tile([BC, 1], FP, tag="m")
                        nc.vector.tensor_scalar_mul(out=m, in0=negm, scalar1=-1.0)
                        nc.scalar.activation(out=alpha, in_=up, func=AF.Identity,
                                             bias=m, scale=1.0)
                u2 = cp.tile([BC, K], FP, tag="u2")
                nc.vector.tensor_add(out=u2, in0=alpha, in1=stopt)
                negm2 = cp.tile([BC, 1], FP, tag="negm2")
                nc.vector.reduce_max(out=negm2, in_=u2,
                                     axis=mybir.AxisListType.X, negate=True)
                es2 = cp.tile([BC, K], FP, tag="es2")
                sm2 = cp.tile([BC, 1], FP, tag="sm2")
                nc.scalar.activation(out=es2, in_=u2, func=AF.Exp,
                                     bias=negm2, scale=1.0, accum_out=sm2)
                ln2 = cp.tile([BC, 1], FP, tag="ln2")
                nc.scalar.activation(out=ln2, in_=sm2, func=AF.Ln)
                allsc = cp.tile([BC, 1], FP, tag="allsc")
                nc.vector.tensor_sub(out=allsc, in0=ln2, in1=negm2)
                nc.sync.dma_start(out=allsc_out[:, :], in_=allsc)
                nc.sync.dma_start(out=feats_out[:, :], in_=featsT)

    return nc


def _prep_inputs(inp):
    """Host-side prep: embedding gather + transposes. Returns (per_core, shared)."""
    x = inp["embed"][inp["sentence"]]  # [B, T, E] f32
    per_core = []
    for c in range(NCORES):
        xc = x[c * BC:(c + 1) * BC]  # [8, T, E]
        xTc = np.ascontiguousarray(xc.transpose(2, 0, 1).reshape(E, ROWS))
        per_core.append(xTc.astype(np.float32, copy=False))

    sh = {}
    for l, key in ((0, "l0"), (1, "l1")):
        for d in "fb":
            sh[f"w{l}{d}"] = np.ascontiguousarray(inp[f"Wih_{key}{d}"].T)
            sh[f"whh{l}{d}"] = np.ascontiguousarray(inp[f"Whh_{key}{d}"].T)
            sh[f"b{l}{d}"] = (inp[f"bih_{key}{d}"] + inp[f"bhh_{key}{d}"])[None, :]
    sh["gam"] = inp["ln_gamma"][None, :]
    sh["bet"] = inp["ln_beta"][None, :]
    sh["woutT"] = np.ascontiguousarray(inp["Wout"].T)
    sh["boutT"] = inp["bout"][:, None]
    trans = inp["transitions"]
    sh["expTT"] = np.ascontiguousarray(np.exp(trans).T)
    sh["stopR"] = np.repeat(trans[STOP][None, :], BC, axis=0)
    a0 = np.full((BC, K), NEG, np.float32)
    a0[:, START] = 0.0
    sh["alpha0"] = a0
    import ml_dtypes
    sh = {k: np.ascontiguousarray(v.astype(np.float32, copy=False))
          for k, v in sh.items()}
    for key in ("w1f", "w1b", "whh0f", "whh0b", "whh1f", "whh1b"):
        sh[key] = sh[key].astype(ml_dtypes.bfloat16)
    return per_core, sh


def _real_path(feats, tags, trans):
    # feats: [B, T, K]
    b = feats.shape[0]
    tf = np.concatenate([np.full((b, 1), START, tags.dtype), tags], axis=1)
    trans_sc = trans[tf[:, 1:], tf[:, :-1]].sum(axis=1, dtype=np.float32)
    emit_sc = np.take_along_axis(
        feats, tags[:, :, None].astype(np.int64), axis=2)[..., 0].sum(
        axis=1, dtype=np.float32)
    stop_sc = trans[STOP, tags[:, -1]]
    return trans_sc + emit_sc + stop_sc



_runner = None


class _Runner:
    """Persistent jitted SPMD executor for the bass kernel (replicates the
    multi-core path of bass2jax.run_bass_via_pjrt, but keeps the jitted
    callable and device-resident inputs so repeat calls measure execution,
    not retracing/transfers)."""

    def __init__(self, nc):
        import jax
        from jax.experimental.shard_map import shard_map
        from jax.sharding import Mesh, NamedSharding, PartitionSpec
        from concourse import bass2jax, mybir

        bass2jax.install_neuronx_cc_hook()
        self.jax = jax
        self.nc = nc
        partition_name = (nc.partition_id_tensor.name
                          if nc.partition_id_tensor else None)
        in_names, out_names, out_avals, zero_outs = [], [], [], []
        for alloc in nc.m.functions[0].allocations:
            if not isinstance(alloc, mybir.MemoryLocationSet):
                continue
            name = alloc.memorylocations[0].name
            if alloc.kind == "ExternalInput":
                if name != partition_name:
                    in_names.append(name)
            elif alloc.kind == "ExternalOutput":
                out_names.append(name)
                shape = tuple(alloc.tensor_shape)
                dtype = mybir.dt.np(alloc.dtype)
                out_avals.append(jax.core.ShapedArray(shape, dtype))
                zero_outs.append(np.zeros(shape, dtype))
        self.n_params = len(in_names)
        self.in_names = list(in_names)
        self.out_names = out_names
        self.out_avals = out_avals
        self.zero_outs = zero_outs
        all_in_names = in_names + out_names
        if partition_name is not None:
            all_in_names.append(partition_name)

        def _body(*args):
            operands = list(args)
            if partition_name is not None:
                operands.append(bass2jax.partition_id_tensor())
            outs = bass2jax._bass_exec_p.bind(
                *operands,
                out_avals=tuple(out_avals),
                in_names=tuple(all_in_names),
                out_names=tuple(out_names),
                lowering_input_output_aliases=(),
                sim_require_finite=True,
                sim_require_nnan=True,
                nc=nc,
            )
            return tuple(outs)

        devices = jax.devices()[:NCORES]
        self.mesh = Mesh(np.asarray(devices), ("core",))
        n_outs = len(out_avals)
        in_specs = (PartitionSpec("core"),) * (self.n_params + n_outs)
        out_specs = (PartitionSpec("core"),) * n_outs
        donate = tuple(range(self.n_params, self.n_params + n_outs))
        self.sharding = NamedSharding(self.mesh, PartitionSpec("core"))
        self.fn = jax.jit(
            shard_map(_body, mesh=self.mesh, in_specs=in_specs,
                      out_specs=out_specs, check_rep=False),
            donate_argnums=donate, keep_unused=True)
        self.placed = None
        self._zpool = []

    def place_inputs(self, in_maps):
        concat = [
            np.concatenate([np.asarray(in_maps[c][n]) for c in range(NCORES)],
                           axis=0)
            for n in self.in_names
        ]
        self.placed = [self.jax.device_put(a, self.sharding) for a in concat]
        self.jax.block_until_ready(self.placed)

    def _one_zero_set(self):
        zs = [self.jax.device_put(
                  np.zeros((NCORES * z.shape[0], *z.shape[1:]), z.dtype),
                  self.sharding)
              for z in self.zero_outs]
        self.jax.block_until_ready(zs)
        return zs

    def prewarm_zeros(self, n):
        """Pre-place n donated zero-output sets on device so timed
        executions contain no host->device transfers."""
        while len(self._zpool) < n:
            self._zpool.append(self._one_zero_set())

    def execute(self):
        """Run on pre-placed inputs; returns device arrays (no D2H)."""
        zs = self._zpool.pop() if self._zpool else self._one_zero_set()
        outs = self.fn(*self.placed, *zs)
        self.jax.block_until_ready(outs)
        return outs

    def execute_pipelined(self, n):
        """Launch n kernel executions back-to-back without host blocking
        between them (jax async dispatch), block once at the end. Returns
        total seconds. Amortizes the per-call axon RPC/dispatch floor so
        per-iteration time approaches true device execution time."""
        import time as _time
        self.prewarm_zeros(n)
        zsets = [self._zpool.pop() for _ in range(n)]
        t0 = _time.perf_counter()
        outs = [self.fn(*self.placed, *zs) for zs in zsets]
        self.jax.block_until_ready(outs)
        return _time.perf_counter() - t0

    def run(self, in_maps):
        self.place_inputs(in_maps)
        outs = self.execute()
        results = []
        for c in range(NCORES):
            results.append({
                name: np.asarray(outs[i]).reshape(
                    NCORES, *self.out_avals[i].shape)[c]
                for i, name in enumerate(self.out_names)
            })
        return results


def get_runner():
    global _runner, _nc_cache
    if _runner is None:
        if _nc_cache is None:
            nc = build_nc()
            _split_waits(nc)
            _nc_cache = nc
        _runner = _Runner(_nc_cache)
    return _runner


def kernel(**inputs) -> np.ndarray:
    inp = {k: np.asarray(v) for k, v in inputs.items()}
    per_core, sh = _prep_inputs(inp)

    global _nc_cache
    if _nc_cache is None:
        nc = build_nc()
        _split_waits(nc)
        _nc_cache = nc
    nc = _nc_cache

    in_maps = [dict(sh, xT=per_core[c]) for c in range(NCORES)]
    runner = get_runner()
    results = runner.run(in_maps)

    all_sc = np.concatenate(
        [results[c]["allsc_out"][:, 0] for c in range(NCORES)])  # [B]
    feats = np.concatenate(
        [results[c]["feats_out"].reshape(K, BC, T).transpose(1, 2, 0)
         for c in range(NCORES)], axis=0)  # [B, T, K]
    real_sc = _real_path(feats, inp["tags"], inp["transitions"])
    loss = (all_sc - real_sc).mean(dtype=np.float32)
    return np.asarray(loss, dtype=np.float32)


def kernel(**inputs) -> np.ndarray:
    inp = {k: np.asarray(v) for k, v in inputs.items()}
    per_core, sh = _prep_inputs(inp)

    in_maps = [dict(sh, xT=per_core[c]) for c in range(NCORES)]
    runner = get_runner()
    results = runner.run(in_maps)

    all_sc = np.concatenate(
        [results[c]["allsc_out"][:, 0] for c in range(NCORES)])  # [B]
    feats = np.concatenate(
        [results[c]["feats_out"].reshape(K, BC, T).transpose(1, 2, 0)
         for c in range(NCORES)], axis=0)  # [B, T, K]
    real_sc = _real_path(feats, inp["tags"], inp["transitions"])
    loss = (all_sc - real_sc).mean(dtype=np.float32)
    return np.asarray(loss, dtype=np.float32)
